# revision 1
# baseline (speedup 1.0000x reference)
"""ConvBlock (proj -> depthwise causal conv1d -> silu, gated, out-proj) on 8 TRN2 NeuronCores.

Sharding: data-parallel over tokens (B*L = 8192 -> 1024 tokens/core) with a
3-token left halo per shard (zeros at batch starts), so the causal depthwise
conv needs no cross-core communication.

Per-core layout: activations are kept transposed [channels(partitions), tokens]
so the conv is a shifted access pattern along the free dim. All host-side
re-layouts (transposes, chunking) are done in numpy here.

Single-shot latency tuning (vs the first working version):
- PE warm-up: dummy matmuls run during the initial DMA wait so the tensor
  engine's clock-gate ramp (~3us at reduced clock) burns on throwaway work.
- Fast start: e=0 (and e=1) run against split weight tiles with matmuls
  emitted in DMA-arrival order (x_0, gate weights, x_1, remaining proj
  weights, x_2..), so the PE envelope starts as soon as the first chunk
  lands and tracks the serial transfer pipe without stalling.
- Short tail: out-proj results are copied+DMA'd per 512-col chunk (bf16 on
  the wire) so the final DMA after the last matmul is quarter-sized.
"""
import numpy as np

import concourse.bacc as bacc
import concourse.mybir as mybir
import concourse.tile as tile
from concourse.bass_utils import run_bass_kernel_spmd

F32 = mybir.dt.float32
BF16 = mybir.dt.bfloat16
AF = mybir.ActivationFunctionType

B, L, D, E, DC = 2, 4096, 1024, 2048, 4
NCORES = 8
T = B * L // NCORES          # 1024 tokens per core
H = DC - 1                   # 3 halo tokens
TH = T + H                   # 1027
ED = E // 128                # 16 e-chunks
KD = D // 128                # 8 d-chunks (contraction for proj/gate)
DM = D // 128                # 8 output-row chunks
NT = 512                     # matmul moving-dim tile (PSUM bank limit, f32)

# dtype knobs
MM_DT = BF16                 # proj/gate matmul operand dtype
EW_DT = BF16                 # elementwise dtype for val/sv/sg staging
H_DT = BF16                  # h (= out-matmul rhs) dtype; wot matches
Y_DT = BF16                  # y output wire dtype (upcast to f32 on host)

N_WARM = 56  # PE warm-up dummy matmuls (64-col)


def _build_nc(reps=1):
    nc = bacc.Bacc("TRN2", target_bir_lowering=False, debug=False,
                   num_devices=NCORES)

    xt = nc.dram_tensor("xt", [KD, 128, TH], MM_DT, kind="ExternalInput").ap()
    # wpg[e] = [128, 2048]: cols 0:1024 proj lhsT k-tiles, cols 1024:2048 gate
    wpg = nc.dram_tensor("wpg", [ED, 128, 2 * KD * 128], MM_DT,
                         kind="ExternalInput").ap()
    # wot[j] = [128, ED*128]: out-proj lhsT tiles for row-chunk j, all e side by side
    wot = nc.dram_tensor("wot", [DM, 128, ED * 128], H_DT,
                         kind="ExternalInput").ap()
    wcv = nc.dram_tensor("wcv", [128, ED * DC], F32, kind="ExternalInput").ap()
    bcv = nc.dram_tensor("bcv", [128, ED], F32, kind="ExternalInput").ap()
    yt = nc.dram_tensor("yt", [DM, 128, T], Y_DT, kind="ExternalOutput").ap()
    scr = nc.dram_tensor("scr", [128, 2], F32, kind="Internal").ap()

    with tile.TileContext(nc) as tc:
        with tc.tile_pool(name="xp", bufs=1) as xp, \
             tc.tile_pool(name="cp", bufs=1) as cp, \
             tc.tile_pool(name="hp", bufs=1) as hp, \
             tc.tile_pool(name="w0", bufs=1) as w0, \
             tc.tile_pool(name="wp", bufs=3) as wp, \
             tc.tile_pool(name="vp", bufs=3) as vp, \
             tc.tile_pool(name="tp", bufs=3) as tp, \
             tc.tile_pool(name="wo", bufs=5) as wopool, \
             tc.tile_pool(name="yp", bufs=3) as yp, \
             tc.tile_pool(name="ps", bufs=1, space="PSUM") as ps:

            # --- PE warm-up: keep the tensor engine busy (and its clock
            # ramping) on garbage matmuls while the first DMAs land.
            dmm = cp.tile([128, 64], MM_DT, name="dmm")
            nc.gpsimd.memset(dmm[:], 0.0)
            pw = ps.tile([128, NT], F32, name="pw", tag="yb", bufs=2)
            for _ in range(N_WARM):
                nc.tensor.matmul(pw[:64, :64], dmm[:], dmm[:],
                                 start=True, stop=True)

            # --- initial DMA queue, ordered so e=0's consumption tracks
            # serial transfer arrival: tiny k=0 proj weights, x chunk 0,
            # all gate weights, x chunk 1, remaining proj weights, the rest
            # of x, then conv params and the e=1 weight halves.
            wpgA0 = w0.tile([128, 128], MM_DT, name="wpgA0")
            nc.sync.dma_start(wpgA0[:], wpg[0][:, 0:128])
            xt_sb = [None] * KD
            def dma_x(k):
                x_k = xp.tile([128, TH], MM_DT, name=f"x_{k}")
                nc.sync.dma_start(x_k[:], xt[k])
                xt_sb[k] = x_k
            dma_x(0)
            wpgB = w0.tile([128, KD * 128], MM_DT, name="wpgB")
            nc.sync.dma_start(wpgB[:], wpg[0][:, KD * 128:2 * KD * 128])
            dma_x(1)
            # first two proj k-tiles of e=1: lets the PE fill x_2-arrival
            # stalls with e=1 work borrowed into the idle yb PSUM banks
            wpg1a0 = w0.tile([128, 256], MM_DT, name="wpg1a0")
            nc.sync.dma_start(wpg1a0[:], wpg[1][:, 0:256])
            wpgA1 = w0.tile([128, (KD - 1) * 128], MM_DT, name="wpgA1")
            nc.sync.dma_start(wpgA1[:], wpg[0][:, 128:KD * 128])
            for k in range(2, KD):
                dma_x(k)
            wcv_sb = cp.tile([128, ED * DC], F32, name="wcv_sb")
            nc.sync.dma_start(wcv_sb[:], wcv[:])
            bcv_sb = cp.tile([128, ED], F32, name="bcv_sb")
            nc.sync.dma_start(bcv_sb[:], bcv[:])
            wpg1a = w0.tile([128, KD * 128], MM_DT, name="wpg1a")
            nc.sync.dma_start(wpg1a[:], wpg[1][:, 0:KD * 128])
            wpg1b = w0.tile([128, KD * 128], MM_DT, name="wpg1b")
            nc.sync.dma_start(wpg1b[:], wpg[1][:, KD * 128:2 * KD * 128])
            # warm the ACT Silu table while initial DMAs are in flight
            silu_warm = cp.tile([128, 2], F32, name="silu_warm")
            nc.gpsimd.memset(silu_warm[:], 0.0)
            nc.scalar.activation(silu_warm[:], silu_warm[:], AF.Silu)

            h_all = hp.tile([128, ED * T], H_DT, name="h_all")

            val_cols = [(0, NT), (NT, NT), (2 * NT, TH - 2 * NT)]

            def xwin(k, c0, w):
                return xt_sb[k][:, c0:c0 + w]

            def epilogue(e, pv, pg):
                # stage val (PSUM -> SBUF) on ACT
                val_sb = vp.tile([128, TH], EW_DT, name="val_sb", tag="val")
                for n, (c0, w) in enumerate(val_cols):
                    nc.scalar.copy(val_sb[:, c0:c0 + w], pv[n][:, :w])
                # silu(gate) (PSUM -> SBUF) on ACT
                sg = tp.tile([128, T], EW_DT, name="sg", tag="sg")
                for n in range(2):
                    nc.scalar.activation(sg[:, n * NT:(n + 1) * NT], pg[n][:],
                                         AF.Silu)

                # depthwise causal conv: acc = sum_k wc[:,k] * val[:, k:k+T]
                acc = tp.tile([128, T], EW_DT, name="acc", tag="acc")
                nc.vector.tensor_scalar_mul(acc[:], val_sb[:, 0:T],
                                            wcv_sb[:, e * DC:e * DC + 1])
                for kk in range(1, DC):
                    m = tp.tile([128, T], EW_DT, name="m", tag="m")
                    nc.vector.tensor_scalar_mul(
                        m[:], val_sb[:, kk:kk + T],
                        wcv_sb[:, e * DC + kk:e * DC + kk + 1])
                    nc.vector.tensor_add(acc[:], acc[:], m[:])
                # silu(conv + b_conv) on ACT
                sv = tp.tile([128, T], EW_DT, name="sv", tag="sv")
                nc.scalar.activation(sv[:], acc[:], AF.Silu,
                                     bias=bcv_sb[:, e:e + 1])
                # h = silu(v) * silu(gate)
                nc.vector.tensor_mul(h_all[:, e * T:(e + 1) * T], sv[:], sg[:])

            def alloc_psum():
                pv = []
                for n, (c0, w) in enumerate(val_cols):
                    pv.append(ps.tile([128, NT], F32, name=f"pv{n}",
                                      tag=f"pv{n}", bufs=(2 if n == 0 else 1)))
                pg = [ps.tile([128, NT], F32, name=f"pg{n}", tag=f"pg{n}")
                      for n in range(2)]
                return pv, pg

            e1_pv01 = []

            def phase_a0(stP, stG, e1early=False):
                # e=0: emission tracks DMA arrival order (x_0, gate weights,
                # x_1, e1-proj head, remaining proj weights, x_2, ...) so the
                # PE envelope starts as soon as x_0 lands and never stalls.
                pv, pg = alloc_psum()
                seen = {}

                def flags(bank):
                    i = seen.get(bank, 0)
                    seen[bank] = i + 1
                    return {"start": i == 0, "stop": i == KD - 1}

                def val_n(n, k):
                    c0, w = val_cols[n]
                    nc.tensor.matmul(pv[n][:, :w], stP(k), xwin(k, c0, w),
                                     **flags(f"v{n}"))

                def gate_n(n, k):
                    c0 = H + n * NT
                    nc.tensor.matmul(pg[n][:], stG(k), xwin(k, c0, NT),
                                     **flags(f"g{n}"))

                # consumption tracks arrival: x_0, B, x_1, wpg1a0, A1, x_2...
                val_n(0, 0)            # needs x_0 + A0
                val_n(1, 0)
                val_n(2, 0)
                gate_n(0, 0)           # needs x_0 + B
                gate_n(1, 0)
                gate_n(0, 1)           # needs x_1 + B
                gate_n(1, 1)
                if e1early:
                    # e=1's first two proj k-steps into the idle yb banks;
                    # accumulation continues in phase_a1_rest (start=False)
                    p0 = ps.tile([128, NT], F32, name="e1v0", tag="yb",
                                 bufs=2)
                    p1 = ps.tile([128, NT], F32, name="e1v1", tag="yb",
                                 bufs=2)
                    for k in (0, 1):
                        st = wpg1a0[:, k * 128:(k + 1) * 128]
                        nc.tensor.matmul(p0[:], st, xwin(k, 0, NT),
                                         start=(k == 0), stop=False)
                        nc.tensor.matmul(p1[:], st, xwin(k, NT, NT),
                                         start=(k == 0), stop=False)
                    e1_pv01[:] = [p0, p1]
                val_n(0, 1)            # needs x_1 + A1
                val_n(1, 1)
                val_n(2, 1)
                for k in range(2, KD):
                    val_n(0, k)
                    val_n(1, k)
                    val_n(2, k)
                    gate_n(0, k)
                    gate_n(1, k)
                epilogue(0, pv, pg)

            def phase_a1_rest(stP, stG):
                # e=1 with its first two proj k-steps already accumulated in
                # the borrowed yb banks during phase_a0
                pv = [e1_pv01[0], e1_pv01[1],
                      ps.tile([128, NT], F32, name="pv2", tag="pv2")]
                pg = [ps.tile([128, NT], F32, name=f"pg{n}", tag=f"pg{n}")
                      for n in range(2)]
                for n in range(2):
                    for k in range(2, KD):
                        nc.tensor.matmul(
                            pv[n][:], stP(k), xwin(k, val_cols[n][0], NT),
                            start=False, stop=(k == KD - 1))
                c0, w = val_cols[2]
                for k in range(KD):
                    nc.tensor.matmul(pv[2][:, :w], stP(k), xwin(k, c0, w),
                                     start=(k == 0), stop=(k == KD - 1))
                for n in range(2):
                    c0 = H + n * NT
                    for k in range(KD):
                        nc.tensor.matmul(
                            pg[n][:], stG(k), xwin(k, c0, NT),
                            start=(k == 0), stop=(k == KD - 1))
                epilogue(1, pv, pg)

            def phase_a(e, stP, stG):
                pv, pg = alloc_psum()
                for n, (c0, w) in enumerate(val_cols):
                    for k in range(KD):
                        nc.tensor.matmul(
                            pv[n][:, :w], stP(k), xwin(k, c0, w),
                            start=(k == 0), stop=(k == KD - 1))
                for n in range(2):
                    c0 = H + n * NT
                    for k in range(KD):
                        nc.tensor.matmul(
                            pg[n][:], stG(k), xwin(k, c0, NT),
                            start=(k == 0), stop=(k == KD - 1))
                epilogue(e, pv, pg)

            wot_tiles = [None] * DM

            def phase_b(j):
                if wot_tiles[j] is None:
                    wo_t = wopool.tile([128, ED * 128], H_DT, name="wo_t",
                                       tag="wo", bufs=DM)
                    nc.sync.dma_start(wo_t[:], wot[j])
                    wot_tiles[j] = wo_t
                wo_t = wot_tiles[j]
                for n in range(2):
                    yb = ps.tile([128, NT], F32, name="yb", tag="yb", bufs=2)
                    for e in range(ED):
                        nc.tensor.matmul(
                            yb[:], wo_t[:, e * 128:(e + 1) * 128],
                            h_all[:, e * T + n * NT: e * T + (n + 1) * NT],
                            start=(e == 0), stop=(e == ED - 1))
                    y_out = yp.tile([128, NT], Y_DT, name="y_out", tag="yo")
                    nc.scalar.copy(y_out[:], yb[:])
                    nc.sync.dma_start(yt[j][:, n * NT:(n + 1) * NT], y_out[:])

            # proj/gate weights are rep-invariant: DMA each e-chunk once
            # (rep 0) and keep it resident in SBUF for later reps
            wpg_tiles = [None] * ED
            for rep in range(reps):
                for e in range(ED):
                    if e == 0:
                        phase_a0(
                            lambda k: wpgA0[:] if k == 0
                            else wpgA1[:, (k - 1) * 128:k * 128],
                            lambda k: wpgB[:, k * 128:(k + 1) * 128],
                            e1early=(rep == 0))
                        continue
                    if e == 1:
                        stP1 = lambda k: wpg1a[:, k * 128:(k + 1) * 128]
                        stG1 = lambda k: wpg1b[:, k * 128:(k + 1) * 128]
                        if rep == 0:
                            phase_a1_rest(stP1, stG1)
                        else:
                            phase_a(1, stP1, stG1)
                        continue
                    if rep == 0:
                        wpg_t = wp.tile([128, 2 * KD * 128], MM_DT,
                                        name="wpg_t", tag="wpg", bufs=ED - 2)
                        nc.sync.dma_start(wpg_t[:], wpg[e])
                        wpg_tiles[e] = wpg_t
                    else:
                        wpg_t = wpg_tiles[e]
                    phase_a(e,
                            lambda k: wpg_t[:, k * 128:(k + 1) * 128],
                            lambda k: wpg_t[:, (KD + k) * 128:(KD + k + 1) * 128])
                for j in range(DM):
                    phase_b(j)
            # tiny trailing transfer so the last y DMA's completion
            # semaphore posts without waiting a full engine slot
            nc.sync.dma_start(scr[:], silu_warm[:])

    nc.compile()
    return nc


_NC_CACHE = {}


def _get_nc():
    if "nc" not in _NC_CACHE:
        _NC_CACHE["nc"] = _build_nc()
    return _NC_CACHE["nc"]


def make_in_maps(x, W_proj, W_gate, W_conv, b_conv, W_out):
    """Host-side sharding + re-layout into per-core input dicts."""
    mm_np = mybir.dt.np(MM_DT)
    h_np = mybir.dt.np(H_DT)
    xf = np.ascontiguousarray(np.asarray(x, dtype=np.float32).reshape(B * L, D))
    # lhsT tile layouts: wpt[e, kp, k*128+m] = W_proj[e*128+m, k*128+kp]
    wpt = (np.asarray(W_proj, np.float32).reshape(ED, 128, KD, 128)
           .transpose(0, 3, 2, 1).reshape(ED, 128, KD * 128))
    wgt = (np.asarray(W_gate, np.float32).reshape(ED, 128, KD, 128)
           .transpose(0, 3, 2, 1).reshape(ED, 128, KD * 128))
    wpg = np.ascontiguousarray(
        np.concatenate([wpt, wgt], axis=2).astype(mm_np))
    # wot[j, p, e*128+m] = W_out[j*128+m, e*128+p]
    wot = np.ascontiguousarray(
        np.asarray(W_out, np.float32).reshape(DM, 128, ED, 128)
        .transpose(0, 3, 2, 1).reshape(DM, 128, ED * 128).astype(h_np))
    # wcv[p, e*DC+k] = W_conv[e*128+p, 0, k]; bcv[p, e] = b_conv[e*128+p]
    wcv = np.ascontiguousarray(
        np.asarray(W_conv, np.float32).reshape(ED, 128, DC)
        .transpose(1, 0, 2).reshape(128, ED * DC))
    bcv = np.ascontiguousarray(
        np.asarray(b_conv, np.float32).reshape(ED, 128).T)

    in_maps = []
    for c in range(NCORES):
        s = c * T
        hx = np.zeros((TH, D), dtype=np.float32)
        if s % L != 0:
            hx[0:H] = xf[s - H:s]
        hx[H:H + T] = xf[s:s + T]
        xt = np.ascontiguousarray(hx.T.reshape(KD, 128, TH).astype(mm_np))
        in_maps.append({"xt": xt, "wpg": wpg, "wot": wot,
                       "wcv": wcv, "bcv": bcv})
    return in_maps


def assemble_output(results):
    out = np.empty((B * L, D), dtype=np.float32)
    for c in range(NCORES):
        yt = results[c]["yt"]              # [DM, 128, T]
        out[c * T:(c + 1) * T] = yt.astype(np.float32).reshape(D, T).T
    return out.reshape(B, L, D)


def kernel(x, W_proj, W_gate, W_conv, b_conv, W_out):
    nc = _get_nc()
    in_maps = make_in_maps(x, W_proj, W_gate, W_conv, b_conv, W_out)
    res = run_bass_kernel_spmd(nc, in_maps, core_ids=list(range(NCORES)))
    return assemble_output(res.results)

